# revision 1
# baseline (speedup 1.0000x reference)
"""GCN message-passing kernel for Trainium2 (8 NeuronCores, SPMD).

Math (matches the reference):
    gf   = RF @ W_g                          (2048, 3)   gate features
    H_k  = RF @ W_k                          (2048, 4096) per edge type k in {0,1,2}
    gate(e) = sigmoid(gf[src_e, k_e] + b_glab[p_e])
    upd[t]  = sum_{e->t} gate(e) * (H_{k_e}[src_e] + b_lab[p_e])
    out  = relu(upd)

Because every image's graph is self-contained (32 regions/image) the
edge aggregation is a block-diagonal linear operator: with 4 images per
128-row block,
    upd = sum_k M_k @ H_k + G @ b_lab
where M_k are (128x128)-block-diagonal gate matrices and G is (2048,81).
M_k / G are built ON DEVICE from gf with one-hot constant matrices (host
only prepares 0/1 index matrices), so all data-dependent FLOPs run on
Trainium.

Sharding: the output D dim (4096) is split 8 ways -> each core computes
all 2048 rows x its 512 columns, holding a (4096 x 3*512) slice of
W_conv.  This avoids replicating the 201MB W_conv read (per-core DMA is
~36MB vs ~210MB for image-sharding).  No collectives needed; host
concatenates the column slices.
"""

import numpy as np
import ml_dtypes

# problem constants (hardcoded per contract)
N_IMG = 64
REG = 32
RPI = 32
NUM_REL = 20
D = 4096
NPRED = 81
N = N_IMG * REG          # 2048
NCORES = 8
CW = D // NCORES         # 512 output cols per core
NBLK = N // 128          # 16 row blocks
IPB = 128 // REG         # 4 images per block
EPB = IPB * NUM_REL      # 80 edges per block per edge type

BF = ml_dtypes.bfloat16

_prog_cache = {}


def _build_program():
    import concourse.bass as bass
    import concourse.tile as tile
    from concourse import bacc, mybir

    bf16 = mybir.dt.bfloat16
    f32 = mybir.dt.float32
    AF = mybir.ActivationFunctionType
    ALU = mybir.AluOpType

    nc = bacc.Bacc("TRN2", target_bir_lowering=False, debug=False,
                   num_devices=NCORES)

    rft = nc.dram_tensor("rft", [NBLK, 128, 32 * 128], bf16, kind="ExternalInput").ap()
    w = nc.dram_tensor("w", [128, 3 * 32 * CW], bf16, kind="ExternalInput").ap()
    wg = nc.dram_tensor("wg", [128, 32 * 3], bf16, kind="ExternalInput").ap()
    blab = nc.dram_tensor("blab", [NPRED, CW], bf16, kind="ExternalInput").ap()
    bgb = nc.dram_tensor("bgb", [128, NPRED], bf16, kind="ExternalInput").ap()
    srct = nc.dram_tensor("srct", [128, NBLK * 2 * EPB], bf16, kind="ExternalInput").ap()
    srco = nc.dram_tensor("srco", [EPB, NBLK * 2 * 128], bf16, kind="ExternalInput").ap()
    tgto = nc.dram_tensor("tgto", [EPB, NBLK * 2 * 128], bf16, kind="ExternalInput").ap()
    p1h = nc.dram_tensor("p1h", [EPB, NBLK * NPRED], bf16, kind="ExternalInput").ap()
    p1hs = nc.dram_tensor("p1hs", [128, NPRED], bf16, kind="ExternalInput").ap()
    ident = nc.dram_tensor("ident", [128, 128], bf16, kind="ExternalInput").ap()
    out = nc.dram_tensor("out", [NBLK, 128, CW], f32, kind="ExternalOutput").ap()

    DEPTH = 5  # blocks in the startup phase (absorbs the W-slice DMA time)
    with tile.TileContext(nc) as tc:
        with (
            tc.tile_pool(name="consts", bufs=1) as cpool,
            tc.tile_pool(name="rft", bufs=DEPTH + 2) as rpool,
            tc.tile_pool(name="deep", bufs=DEPTH + 1) as dpool,
            tc.tile_pool(name="small", bufs=2) as spool,
            tc.tile_pool(name="osb", bufs=2) as opool,
            tc.tile_pool(name="ph", bufs=2, space="PSUM") as php,
            tc.tile_pool(name="pgf", bufs=2, space="PSUM") as pgfp,
            tc.tile_pool(name="prg", bufs=1, space="PSUM") as prgp,
            tc.tile_pool(name="pgt", bufs=1, space="PSUM") as pgtp,
            tc.tile_pool(name="pmt", bufs=1, space="PSUM") as pmtp,
            tc.tile_pool(name="pout", bufs=1, space="PSUM") as poutp,
        ):
            # --- input DMAs, ordered by when the PE needs each tensor ---
            wg_sb = cpool.tile([128, 32 * 3], bf16, tag="wg")
            nc.sync.dma_start(out=wg_sb[:], in_=wg[:])
            WCH = 4 * CW  # w0 chunk: 4 d-tiles
            w0_ch = [cpool.tile([128, WCH], bf16, tag=f"w0c{i}", name=f"w0c{i}")
                     for i in range(8)]
            w_sb_k = [None]
            for k in (1, 2):
                wk = cpool.tile([128, 32 * CW], bf16, tag=f"w{k}")
                w_sb_k.append(wk)

            def _rft_half(b, h):
                t = rpool.tile([128, 16 * 128], bf16, tag=f"rft{h}",
                               name=f"rft{h}_{b}")
                nc.sync.dma_start(out=t[:],
                                  in_=rft[b, :, h * 16 * 128:(h + 1) * 16 * 128])
                return t

            def _load_rft(b):
                rft_tiles[b] = [_rft_half(b, 0), _rft_half(b, 1)]

            rft_tiles = {}
            nc.sync.dma_start(out=w0_ch[0][:], in_=w[:, 0:WCH])
            _load_rft(0)
            for i in range(1, 8):
                nc.sync.dma_start(out=w0_ch[i][:],
                                  in_=w[:, i * WCH:(i + 1) * WCH])
            for b in range(1, DEPTH):
                _load_rft(b)
            nc.sync.dma_start(out=w_sb_k[1][:], in_=w[:, 32 * CW:2 * 32 * CW])
            blab_sb = cpool.tile([NPRED, CW], bf16, tag="blab")
            nc.sync.dma_start(out=blab_sb[:], in_=blab[:])
            bgb_sb = cpool.tile([128, NPRED], bf16, tag="bgb")
            nc.sync.dma_start(out=bgb_sb[:], in_=bgb[:])
            srct_sb = cpool.tile([128, NBLK * 2 * EPB], bf16, tag="srct")
            nc.sync.dma_start(out=srct_sb[:], in_=srct[:])
            srco_sb = cpool.tile([EPB, NBLK * 2 * 128], bf16, tag="srco")
            nc.sync.dma_start(out=srco_sb[:], in_=srco[:])
            tgto_sb = cpool.tile([EPB, NBLK * 2 * 128], bf16, tag="tgto")
            nc.sync.dma_start(out=tgto_sb[:], in_=tgto[:])
            p1h_sb = cpool.tile([EPB, NBLK * NPRED], bf16, tag="p1h")
            nc.sync.dma_start(out=p1h_sb[:], in_=p1h[:])
            p1hs_sb = cpool.tile([128, NPRED], bf16, tag="p1hs")
            nc.sync.dma_start(out=p1hs_sb[:], in_=p1hs[:])
            ident_sb = cpool.tile([128, 128], bf16, tag="ident")
            nc.sync.dma_start(out=ident_sb[:], in_=ident[:])
            nc.sync.dma_start(out=w_sb_k[2][:], in_=w[:, 2 * 32 * CW:3 * 32 * CW])

            h_sb, gf_tiles, mtgt, g2_tiles = {}, {}, {}, {}

            def rft_lhsT(b, d):
                return rft_tiles[b][d // 16][:, (d % 16) * 128:(d % 16 + 1) * 128]

            from concourse.tile_rust import add_dep_helper

            def k0_pass(b):
                """H_0(b) = RF_b @ W_0 with gf(b) interleaved.  The gf
                matmul reuses the H matmul's stationary operand (same rft
                tile) via ldweights=False; the local ordering chain keeps
                each gf adjacent to its H partner so the weights are
                still loaded when it executes."""
                ph_t = php.tile([128, CW], f32, tag="ph", name=f"ph{b}_0")
                pgf_t = pgfp.tile([128, 3], f32, tag="pgf", name=f"pgf{b}")
                prev = None
                for d in range(32):
                    lhsT = rft_lhsT(b, d)
                    nc.tensor.matmul(ph_t[:], lhsT,
                                     w0_ch[d // 4][:, (d % 4) * CW:(d % 4 + 1) * CW],
                                     start=(d == 0), stop=(d == 31))
                    h_inst = nc.main_func.blocks[-1].instructions[-1]
                    assert h_inst.opcode == "Matmult"
                    if prev is not None:
                        add_dep_helper(h_inst, prev, sync=False,
                                       reason="k0-chain")
                    nc.tensor.matmul(pgf_t[:], lhsT,
                                     wg_sb[:, d * 3:(d + 1) * 3],
                                     start=(d == 0), stop=(d == 31))
                    gf_inst = nc.main_func.blocks[-1].instructions[-1]
                    assert gf_inst.opcode == "Matmult"
                    gf_inst.ldweights = False
                    add_dep_helper(gf_inst, h_inst, sync=False,
                                   reason="k0-pair")
                    prev = gf_inst
                hk = dpool.tile([128, CW], bf16, tag="h0", name=f"h{b}_0")
                nc.vector.tensor_copy(out=hk[:], in_=ph_t[:])
                h_sb[(b, 0)] = hk
                gf_sb = dpool.tile([128, 3], f32, tag="gf", name=f"gf{b}")
                nc.vector.tensor_copy(out=gf_sb[:], in_=pgf_t[:])
                gf_tiles[b] = gf_sb

            def h_pass(b, k):
                """H_k(b) = RF_b @ W_k for k in (1, 2).  For k=2 the
                self-loop gate is folded into the PSUM->SBUF copy:
                h2s = diag(g2) @ H_2, so stage3 needs no M_2 matmul."""
                tagp = "h1d" if k == 1 else "h2"
                pool = dpool if k == 1 else spool
                ph_t = php.tile([128, CW], f32, tag="ph", name=f"ph{b}_{k}")
                for d in range(32):
                    nc.tensor.matmul(ph_t[:], rft_lhsT(b, d),
                                     w_sb_k[k][:, d * CW:(d + 1) * CW],
                                     start=(d == 0), stop=(d == 31))
                hk = pool.tile([128, CW], bf16, tag=tagp, name=f"h{b}_{k}")
                if k == 2:
                    nc.vector.tensor_scalar_mul(hk[:], ph_t[:], g2_tiles[b][:])
                else:
                    nc.vector.tensor_copy(out=hk[:], in_=ph_t[:])
                h_sb[(b, k)] = hk

            def build(b):
                """Gates -> block-diagonal M_k (lhsT form) and G^T."""
                gf_sb = gf_tiles[b]
                sig = []
                for k in range(2):
                    sg = spool.tile([128, NPRED], bf16, tag=f"sig{k}",
                                    name=f"sig{b}_{k}")
                    nc.scalar.activation(sg[:], bgb_sb[:], AF.Sigmoid,
                                         bias=gf_sb[:, k:k + 1])
                    sig.append(sg)
                g2 = dpool.tile([128, 1], f32, tag="g2", name=f"g2_{b}")
                nc.scalar.activation(g2[:], bgb_sb[:, 0:1], AF.Sigmoid,
                                     bias=gf_sb[:, 2:3])
                g2_tiles[b] = g2

                mt_sb = dpool.tile([128, 3 * 128], bf16, tag="mt",
                                   name=f"mt{b}")
                pgt_t = pgtp.tile([NPRED, 128], f32, tag="pgt", name=f"pgt{b}")
                for k in range(2):
                    prg_t = prgp.tile([EPB, NPRED], f32, tag="prg",
                                      name=f"prg{b}_{k}")
                    nc.tensor.matmul(
                        prg_t[:],
                        srct_sb[:, (b * 2 + k) * EPB:(b * 2 + k + 1) * EPB],
                        sig[k][:], start=True, stop=True)
                    pg = spool.tile([EPB, NPRED], bf16, tag="pg",
                                    name=f"pg{b}_{k}")
                    nc.vector.tensor_mul(
                        pg[:], prg_t[:],
                        p1h_sb[:, b * NPRED:(b + 1) * NPRED])
                    nc.tensor.matmul(
                        pgt_t[:], pg[:],
                        tgto_sb[:, (b * 2 + k) * 128:(b * 2 + k + 1) * 128],
                        start=(k == 0), stop=False)
                    gcol = spool.tile([EPB, 1], f32, tag="gcol",
                                      name=f"gcol{b}_{k}")
                    nc.vector.tensor_reduce(gcol[:], pg[:],
                                            axis=mybir.AxisListType.X,
                                            op=ALU.add)
                    srcg = spool.tile([EPB, 128], bf16, tag="srcg",
                                      name=f"srcg{b}_{k}")
                    nc.vector.tensor_scalar_mul(
                        srcg[:],
                        srco_sb[:, (b * 2 + k) * 128:(b * 2 + k + 1) * 128],
                        gcol[:])
                    pmt_t = pmtp.tile([128, 128], f32, tag="pmt",
                                      name=f"pmt{b}_{k}")
                    nc.tensor.matmul(
                        pmt_t[:], srcg[:],
                        tgto_sb[:, (b * 2 + k) * 128:(b * 2 + k + 1) * 128],
                        start=True, stop=True)
                    nc.vector.tensor_copy(out=mt_sb[:, k * 128:(k + 1) * 128],
                                          in_=pmt_t[:])
                # self-loop: M_2 = diag(g2); G row 0 += g2
                pg2 = spool.tile([128, NPRED], bf16, tag="pg2", name=f"pg2_{b}")
                nc.vector.tensor_scalar_mul(pg2[:], p1hs_sb[:], g2[:])
                nc.tensor.matmul(pgt_t[:], pg2[:], ident_sb[:],
                                 start=False, stop=True)
                gt_sb = dpool.tile([NPRED, 128], bf16, tag="gt", name=f"gt{b}")
                nc.vector.tensor_copy(out=gt_sb[:], in_=pgt_t[:])
                mtgt[b] = (mt_sb, gt_sb)

            def stage3(b):
                mt_sb, gt_sb = mtgt[b]
                pout_t = poutp.tile([128, CW], f32, tag="pout", name=f"po{b}")
                for k in range(2):
                    nc.tensor.matmul(pout_t[:],
                                     mt_sb[:, k * 128:(k + 1) * 128],
                                     h_sb[(b, k)][:],
                                     start=(k == 0), stop=False)
                nc.tensor.matmul(pout_t[:], gt_sb[:], blab_sb[:],
                                 start=False, stop=True)
                nc.vector.tensor_add(pout_t[:], pout_t[:], h_sb[(b, 2)][:])
                out_sb = opool.tile([128, CW], f32, tag="out", name=f"ob{b}")
                nc.scalar.activation(out_sb[:], pout_t[:], AF.Relu)
                nc.sync.dma_start(out=out[b], in_=out_sb[:])
                del h_sb[(b, 0)], h_sb[(b, 1)], h_sb[(b, 2)]
                del gf_tiles[b], mtgt[b], g2_tiles[b]
                del rft_tiles[b]

            # --- phase A: deep startup pipeline over the first DEPTH blocks
            # (k=0 for all of them runs while W[k=1,2] + gate consts stream) ---
            for b in range(DEPTH):
                k0_pass(b)
            for b in range(DEPTH):
                h_pass(b, 1)
                build(b)
            _load_rft(DEPTH)
            if DEPTH + 1 < NBLK:
                _load_rft(DEPTH + 1)
            for b in range(DEPTH):
                h_pass(b, 2)
                stage3(b)

            # --- phase B: W resident; simple per-block processing ---
            for b in range(DEPTH, NBLK):
                if b + 2 < NBLK:
                    _load_rft(b + 2)
                k0_pass(b)
                h_pass(b, 1)
                build(b)
                h_pass(b, 2)
                stage3(b)

    nc.compile()
    return nc


def _host_prep(inputs):
    rf = np.asarray(inputs["region_feats"], dtype=np.float32)
    W = np.asarray(inputs["W_conv"], dtype=np.float32)
    Wg = np.asarray(inputs["W_g"], dtype=np.float32)
    blab = np.asarray(inputs["b_lab"], dtype=np.float32)
    bglab = np.asarray(inputs["b_glab"], dtype=np.float32)
    rels = np.asarray(inputs["rels"])
    preds = np.asarray(inputs["pred_classes"])

    rels_r = rels.reshape(N_IMG, RPI, 3)[:, :NUM_REL].reshape(-1, 3)
    preds_r = preds.reshape(N_IMG, RPI)[:, :NUM_REL].reshape(-1)

    # RF^T tiles: rft_h[b, p, d*128+j] = RF[b*128+j, d*128+p]
    rft_h = np.ascontiguousarray(
        rf.T.reshape(32, 128, NBLK, 128).transpose(2, 1, 0, 3), dtype=BF
    ).reshape(NBLK, 128, 32 * 128)

    # W slices per core: w_h[p, ((k*32+d)*CW)+j] = W[d*128+p, k*D + c*CW + j]
    Wr = W.reshape(32, 128, 3, NCORES, CW)
    w_cores = [
        np.ascontiguousarray(Wr[:, :, :, c, :].transpose(1, 2, 0, 3),
                             dtype=BF).reshape(128, 3 * 32 * CW)
        for c in range(NCORES)
    ]
    wg_h = np.ascontiguousarray(
        Wg.reshape(32, 128, 3).transpose(1, 0, 2), dtype=BF
    ).reshape(128, 32 * 3)
    blab_cores = [
        np.ascontiguousarray(blab[:, c * CW:(c + 1) * CW], dtype=BF)
        for c in range(NCORES)
    ]
    bgb_h = np.ascontiguousarray(
        np.repeat(bglab.reshape(1, NPRED), 128, axis=0), dtype=BF)

    srct_h = np.zeros((128, NBLK * 2 * EPB), np.float32)
    srco_h = np.zeros((EPB, NBLK * 2 * 128), np.float32)
    tgto_h = np.zeros((EPB, NBLK * 2 * 128), np.float32)
    p1h_h = np.zeros((EPB, NBLK * NPRED), np.float32)
    e = np.arange(EPB)
    for b in range(NBLK):
        eb = rels_r[b * EPB:(b + 1) * EPB]
        pb = preds_r[b * EPB:(b + 1) * EPB]
        s = eb[:, 1] - b * 128
        o = eb[:, 2] - b * 128
        # k=0: obj -> subj (src=o, tgt=s); k=1: subj -> obj (src=s, tgt=o)
        srct_h[o, (b * 2 + 0) * EPB + e] = 1.0
        srct_h[s, (b * 2 + 1) * EPB + e] = 1.0
        srco_h[e, (b * 2 + 0) * 128 + o] = 1.0
        srco_h[e, (b * 2 + 1) * 128 + s] = 1.0
        tgto_h[e, (b * 2 + 0) * 128 + s] = 1.0
        tgto_h[e, (b * 2 + 1) * 128 + o] = 1.0
        p1h_h[e, b * NPRED + pb] = 1.0
    p1hs_h = np.zeros((128, NPRED), np.float32)
    p1hs_h[:, 0] = 1.0

    shared = {
        "rft": rft_h,
        "wg": wg_h,
        "bgb": bgb_h,
        "srct": srct_h.astype(BF),
        "srco": srco_h.astype(BF),
        "tgto": tgto_h.astype(BF),
        "p1h": p1h_h.astype(BF),
        "p1hs": p1hs_h.astype(BF),
        "ident": np.eye(128, dtype=np.float32).astype(BF),
    }
    in_maps = []
    for c in range(NCORES):
        m = dict(shared)
        m["w"] = w_cores[c]
        m["blab"] = blab_cores[c]
        in_maps.append(m)
    return in_maps


def _rels_are_blocked(rels):
    """Check each image's relations reference only that image's regions."""
    rels = np.asarray(rels)
    if rels.shape != (N_IMG * RPI, 3):
        return False
    rels_r = rels.reshape(N_IMG, RPI, 3)[:, :NUM_REL]
    img = np.arange(N_IMG)[:, None]
    lo, hi = img * REG, (img + 1) * REG
    so = rels_r[:, :, 1:3]
    return bool(np.all((so >= lo[:, :, None]) & (so < hi[:, :, None])))


def _numpy_fallback(inputs):
    """Reference-equivalent host computation (only used if the per-image
    relation structure assumption is violated)."""
    rf = np.asarray(inputs["region_feats"], dtype=np.float32)
    W = np.asarray(inputs["W_conv"], dtype=np.float32)
    Wg = np.asarray(inputs["W_g"], dtype=np.float32)
    blab = np.asarray(inputs["b_lab"], dtype=np.float32)
    bglab = np.asarray(inputs["b_glab"], dtype=np.float32)
    rels = np.asarray(inputs["rels"])
    preds = np.asarray(inputs["pred_classes"])
    rels_r = rels.reshape(N_IMG, RPI, 3)[:, :NUM_REL].reshape(-1, 3)
    preds_r = preds.reshape(N_IMG, RPI)[:, :NUM_REL].reshape(-1)
    nf = (rf @ W).reshape(-1, D)
    gfe = (rf @ Wg).reshape(-1)
    s, o = rels_r[:, 1], rels_r[:, 2]
    self_ids = np.arange(N)
    idx = np.concatenate([o * 3 + 0, s * 3 + 1, self_ids * 3 + 2])
    pr = np.concatenate([preds_r, preds_r, np.zeros(N, preds_r.dtype)])
    tgt = np.concatenate([s, o, self_ids])
    gate = 1.0 / (1.0 + np.exp(-(gfe[idx] + bglab[pr, 0])))
    msg = gate[:, None] * (nf[idx] + blab[pr])
    upd = np.zeros((N, D), np.float32)
    np.add.at(upd, tgt, msg)
    return np.maximum(upd, 0.0)


def _run(inputs, trace=False):
    from concourse.bass_utils import run_bass_kernel_spmd

    if "nc" not in _prog_cache:
        _prog_cache["nc"] = _build_program()
    nc = _prog_cache["nc"]
    in_maps = _host_prep(inputs)
    try:
        res = run_bass_kernel_spmd(nc, in_maps, core_ids=list(range(NCORES)),
                                   trace=trace)
    except Exception:
        # transient device errors (e.g. NRT_EXEC_UNIT_UNRECOVERABLE) have
        # been observed to clear on retry
        import time
        time.sleep(5)
        res = run_bass_kernel_spmd(nc, in_maps, core_ids=list(range(NCORES)),
                                   trace=trace)
    out = np.empty((N, D), np.float32)
    for c in range(NCORES):
        out[:, c * CW:(c + 1) * CW] = (
            np.asarray(res.results[c]["out"]).reshape(N, CW))
    return out, res


def kernel(**inputs):
    if not _rels_are_blocked(inputs["rels"]):
        return _numpy_fallback(inputs)
    out, _ = _run(inputs, trace=False)
    return out



# revision 6
# speedup vs baseline: 1.2526x; 1.2526x over previous
"""GCN message-passing kernel for Trainium2 (8 NeuronCores, SPMD).

Math (matches the reference):
    gf   = RF @ W_g                          (2048, 3)   gate features
    H_k  = RF @ W_k                          (2048, 4096) per edge type k in {0,1,2}
    gate(e) = sigmoid(gf[src_e, k_e] + b_glab[p_e])
    upd[t]  = sum_{e->t} gate(e) * (H_{k_e}[src_e] + b_lab[p_e])
    out  = relu(upd)

Key restructuring vs the straightforward kernel: the k=0/1 projections are
only needed for rows that appear as edge *sources*.  Per 128-row block
(4 images x 32 regions) the edges reference ~58 unique sources out of 128.
On the PE, matmul cost is (K-chunks x streamed columns) and is independent
of the stationary operand's column count, so we stream the *gathered unique
source features* (N ~ 460 per 8-block group) against stationary W chunks:

    HsT[ch, u] = sum_d W_k[d, ch] * Xs[u, d]      (W chunk stationary)

then PE-transpose HsT -> Hs[u, ch] and scatter with per-block gate
matrices A_k[u, tgt] built on device.  The self-loop H2 = RF @ W2 (all
rows) runs in the classic orientation with the gf matmuls paired in
(reusing the stationary rft operand), exactly like the reference kernel.

Sharding: output D dim split 8 ways (each core: all 2048 rows x 512 cols).
No collectives; host concatenates column slices.  All data-dependent FLOPs
run on Trainium; the host only prepares 0/1 index matrices and gathers /
transposes input rows (pure data movement).
"""

import numpy as np
import ml_dtypes

# problem constants (hardcoded per contract)
N_IMG = 64
REG = 32
RPI = 32
NUM_REL = 20
D = 4096
NPRED = 81
N = N_IMG * REG          # 2048
NCORES = 8
CW = D // NCORES         # 512 output cols per core
NBLK = N // 128          # 16 row blocks
IPB = 128 // REG         # 4 images per block
EPB = IPB * NUM_REL      # 80 edges per block per edge type

BF = ml_dtypes.bfloat16

_prog_cache = {}


def _structure(rels, preds):
    """Compile-time structure: per-block unique edge sources per edge type,
    greedy grouping of blocks into <=512-column streaming groups."""
    rels_r = np.asarray(rels).reshape(N_IMG, RPI, 3)[:, :NUM_REL].reshape(-1, 3)
    preds_r = np.asarray(preds).reshape(N_IMG, RPI)[:, :NUM_REL].reshape(-1)
    st = {"ub": [[], []], "us": [[], []], "inv": [[], []], "boff": [[], []],
          "U": [0, 0], "groups": [[], []],
          "src": [[], []], "tgt": [[], []], "pred": []}
    for b in range(NBLK):
        eb = rels_r[b * EPB:(b + 1) * EPB]
        st["pred"].append(preds_r[b * EPB:(b + 1) * EPB])
        s = eb[:, 1] - b * 128
        o = eb[:, 2] - b * 128
        # k=0: obj -> subj (src=o, tgt=s); k=1: subj -> obj (src=s, tgt=o)
        for k, (src, tgt) in enumerate(((o, s), (s, o))):
            us, inv = np.unique(src, return_inverse=True)
            st["src"][k].append(src)
            st["tgt"][k].append(tgt)
            st["us"][k].append(us)
            st["inv"][k].append(inv)
            st["boff"][k].append(st["U"][k])
            st["ub"][k].append(len(us))
            st["U"][k] += len(us)
    # greedy group packing: consecutive blocks with total unique cols <= 512
    for k in range(2):
        cur, coff = [], 0
        for b in range(NBLK):
            u = st["ub"][k][b]
            if cur and (st["boff"][k][b] + u - coff) > 512:
                st["groups"][k].append((cur, coff, st["boff"][k][b] - coff))
                cur, coff = [], st["boff"][k][b]
            cur.append(b)
        st["groups"][k].append((cur, coff, st["U"][k] - coff))
    return st


def _build_program(st):
    import concourse.bass as bass
    import concourse.tile as tile
    from concourse import bacc, mybir
    from concourse.tile_rust import add_dep_helper

    bf16 = mybir.dt.bfloat16
    f32 = mybir.dt.float32
    AF = mybir.ActivationFunctionType
    ALU = mybir.AluOpType

    nc = bacc.Bacc("TRN2", target_bir_lowering=False, debug=False,
                   num_devices=NCORES)

    U0, U1 = st["U"]
    rft = nc.dram_tensor("rft", [NBLK, 128, 32 * 128], bf16, kind="ExternalInput").ap()
    w2 = nc.dram_tensor("w2", [128, 32 * CW], bf16, kind="ExternalInput").ap()
    w01 = nc.dram_tensor("w01", [128, 2 * 4 * 32 * 128], bf16, kind="ExternalInput").ap()
    wg = nc.dram_tensor("wg", [128, 32 * 3], bf16, kind="ExternalInput").ap()
    blab = nc.dram_tensor("blab", [NPRED, CW], bf16, kind="ExternalInput").ap()
    bgb = nc.dram_tensor("bgb", [128, NPRED], bf16, kind="ExternalInput").ap()
    srct = nc.dram_tensor("srct", [128, NBLK * 2 * EPB], bf16, kind="ExternalInput").ap()
    tgto = nc.dram_tensor("tgto", [EPB, NBLK * 2 * 128], bf16, kind="ExternalInput").ap()
    p1h = nc.dram_tensor("p1h", [EPB, NBLK * NPRED], bf16, kind="ExternalInput").ap()
    p1hs = nc.dram_tensor("p1hs", [128, NPRED], bf16, kind="ExternalInput").ap()
    ident = nc.dram_tensor("ident", [128, 128], bf16, kind="ExternalInput").ap()
    xst0 = nc.dram_tensor("xst0", [32, 128, U0], bf16, kind="ExternalInput").ap()
    xst1 = nc.dram_tensor("xst1", [32, 128, U1], bf16, kind="ExternalInput").ap()
    sdd0 = nc.dram_tensor("sdd0", [EPB, U0], bf16, kind="ExternalInput").ap()
    sdd1 = nc.dram_tensor("sdd1", [EPB, U1], bf16, kind="ExternalInput").ap()
    out = nc.dram_tensor("out", [NBLK, 128, CW], bf16, kind="ExternalOutput").ap()
    xst = [xst0, xst1]
    sddt = [sdd0, sdd1]

    # flat (k, g) stream order
    allgroups = [(k, g) for k in range(2) for g in range(len(st["groups"][k]))]

    with tile.TileContext(nc) as tc:
        with (
            tc.tile_pool(name="consts", bufs=1) as cpool,
            tc.tile_pool(name="rft", bufs=2) as rpool,
            tc.tile_pool(name="wmat", bufs=8) as wpool,
            tc.tile_pool(name="xst", bufs=12) as xpool,
            tc.tile_pool(name="hst", bufs=6) as hstpool,
            tc.tile_pool(name="hs", bufs=32) as hspool,
            tc.tile_pool(name="h2s", bufs=16) as h2pool,
            tc.tile_pool(name="asb", bufs=32) as apool,
            tc.tile_pool(name="gtsb", bufs=16) as gtpool,
            tc.tile_pool(name="gfsb", bufs=3) as gfpool,
            tc.tile_pool(name="sp", bufs=2) as spool,
            tc.tile_pool(name="osb", bufs=3) as opool,
            tc.tile_pool(name="pbig", bufs=4, space="PSUM") as pbig,
            tc.tile_pool(name="psmall", bufs=4, space="PSUM") as psmall,
        ):
            # ---------------- constant DMAs (front of queue) ----------------
            wg_sb = cpool.tile([128, 32 * 3], bf16, tag="wg")
            nc.sync.dma_start(out=wg_sb[:], in_=wg[:])
            bgb_sb = cpool.tile([128, NPRED], bf16, tag="bgb")
            nc.sync.dma_start(out=bgb_sb[:], in_=bgb[:])
            # w2 in 4 chunk-tiles of 8 kc each; these share the wmat ring
            # with the w01 stationary chunks (w01 k=1 reuses w2's slots
            # after the last H2 pass has consumed them).
            w2_ch = []
            for i in range(4):
                t = wpool.tile([128, 8 * CW], bf16, tag="wmat", name=f"w2c{i}")
                nc.sync.dma_start(out=t[:], in_=w2[:, i * 8 * CW:(i + 1) * 8 * CW])
                w2_ch.append(t)

            rft_tiles = {}

            def _load_rft(b):
                t0 = rpool.tile([128, 16 * 128], bf16, tag="rfta", name=f"rfta{b}")
                nc.sync.dma_start(out=t0[:], in_=rft[b, :, :16 * 128])
                t1 = rpool.tile([128, 16 * 128], bf16, tag="rftb", name=f"rftb{b}")
                nc.sync.dma_start(out=t1[:], in_=rft[b, :, 16 * 128:])
                rft_tiles[b] = (t0, t1)

            def rft_lhsT(b, d):
                return rft_tiles[b][d // 16][:, (d % 16) * 128:(d % 16 + 1) * 128]

            _load_rft(0)
            _load_rft(1)
            srct_sb = cpool.tile([128, NBLK * 2 * EPB], bf16, tag="srct")
            nc.sync.dma_start(out=srct_sb[:], in_=srct[:])
            p1hs_sb = cpool.tile([128, NPRED], bf16, tag="p1hs")
            nc.sync.dma_start(out=p1hs_sb[:], in_=p1hs[:])
            ident_sb = cpool.tile([128, 128], bf16, tag="ident")
            nc.sync.dma_start(out=ident_sb[:], in_=ident[:])
            p1h_sb = cpool.tile([EPB, NBLK * NPRED], bf16, tag="p1h")
            nc.sync.dma_start(out=p1h_sb[:], in_=p1h[:])
            tgto_sb = cpool.tile([EPB, NBLK * 2 * 128], bf16, tag="tgto")
            nc.sync.dma_start(out=tgto_sb[:], in_=tgto[:])
            blab_sb = cpool.tile([NPRED, CW], bf16, tag="blab")
            nc.sync.dma_start(out=blab_sb[:], in_=blab[:])
            sdd_sb = []
            for k in range(2):
                t = cpool.tile([EPB, st["U"][k]], bf16, tag=f"sdd{k}")
                nc.sync.dma_start(out=t[:], in_=sddt[k][:])
                sdd_sb.append(t)

            # W01 stationary chunk tiles, one per (k, m); share wmat ring
            w01_sb = {}

            def _load_w01(k, m):
                t = wpool.tile([128, 32 * 128], bf16, tag="wmat",
                               name=f"w01_{k}_{m}")
                off = (k * 4 + m) * 32 * 128
                nc.sync.dma_start(out=t[:], in_=w01[:, off:off + 32 * 128])
                w01_sb[(k, m)] = t

            # XsT streamed tiles, one per (k, g, kc); DMA'd lazily
            xst_sb = {}

            def _load_xst(k, g, kc):
                _, goff, ug = st["groups"][k][g]
                t = xpool.tile([128, ug], bf16, tag="xst",
                               name=f"xst{k}_{g}_{kc}", padded_shape=[128, 512])
                nc.sync.dma_start(out=t[:], in_=xst[k][kc, :, goff:goff + ug])
                xst_sb[(k, g, kc)] = t

            gf_tiles, g2_tiles, h2s_tiles = {}, {}, {}
            sig_tiles, hs_tiles, a_tiles, gt_tiles = {}, {}, {}, {}

            def h2_pass(b):
                """H2(b) = RF_b @ W2 with gf(b) paired in (shared stationary)."""
                ph_t = pbig.tile([128, CW], f32, tag="pb", name=f"ph2_{b}")
                pgf_t = psmall.tile([128, 3], f32, tag="ps", name=f"pgf{b}")
                prev = None
                for d in range(32):
                    lhsT = rft_lhsT(b, d)
                    nc.tensor.matmul(ph_t[:], lhsT,
                                     w2_ch[d // 8][:, (d % 8) * CW:(d % 8 + 1) * CW],
                                     start=(d == 0), stop=(d == 31))
                    h_inst = nc.main_func.blocks[-1].instructions[-1]
                    assert h_inst.opcode == "Matmult"
                    if prev is not None:
                        add_dep_helper(h_inst, prev, sync=False, reason="h2-chain")
                    nc.tensor.matmul(pgf_t[:], lhsT,
                                     wg_sb[:, d * 3:(d + 1) * 3],
                                     start=(d == 0), stop=(d == 31))
                    gf_inst = nc.main_func.blocks[-1].instructions[-1]
                    assert gf_inst.opcode == "Matmult"
                    gf_inst.ldweights = False
                    add_dep_helper(gf_inst, h_inst, sync=False, reason="h2-pair")
                    prev = gf_inst
                gf_sb = gfpool.tile([128, 3], f32, tag="gf", name=f"gf{b}")
                nc.vector.tensor_copy(out=gf_sb[:], in_=pgf_t[:])
                gf_tiles[b] = gf_sb
                # ACT: sigmoids for this block (run while next block's MMs go)
                sigs = []
                for k in range(2):
                    sg = spool.tile([128, NPRED], bf16, tag=f"sig{k}",
                                    name=f"sig{b}_{k}")
                    nc.scalar.activation(sg[:], bgb_sb[:], AF.Sigmoid,
                                         bias=gf_sb[:, k:k + 1])
                    sigs.append(sg)
                sig_tiles[b] = sigs
                g2 = gfpool.tile([128, 1], f32, tag="g2", name=f"g2_{b}")
                nc.scalar.activation(g2[:], bgb_sb[:, 0:1], AF.Sigmoid,
                                     bias=gf_sb[:, 2:3])
                g2_tiles[b] = g2
                # gated self term -> SBUF (frees the psum bank)
                h2s = h2pool.tile([128, CW], bf16, tag="h2s", name=f"h2s{b}")
                nc.vector.tensor_scalar_mul(h2s[:], ph_t[:], g2[:])
                h2s_tiles[b] = h2s

            def build_a(b):
                """Stage A: per-edge gate columns for block b (prg matmuls
                + DVE chain).  PE ops here only depend on sig(b) (ready)."""
                pre = {}
                for k in range(2):
                    prg_t = psmall.tile([EPB, NPRED], f32, tag="ps",
                                        name=f"prg{b}_{k}")
                    nc.tensor.matmul(
                        prg_t[:],
                        srct_sb[:, (b * 2 + k) * EPB:(b * 2 + k + 1) * EPB],
                        sig_tiles[b][k][:], start=True, stop=True)
                    pg = spool.tile([EPB, NPRED], bf16, tag="pg",
                                    name=f"pg{b}_{k}", bufs=3)
                    nc.vector.tensor_mul(
                        pg[:], prg_t[:], p1h_sb[:, b * NPRED:(b + 1) * NPRED])
                    gcol = spool.tile([EPB, 1], f32, tag="gcol",
                                      name=f"gcol{b}_{k}")
                    nc.vector.tensor_reduce(gcol[:], pg[:],
                                            axis=mybir.AxisListType.X,
                                            op=ALU.add)
                    # per-edge gated target one-hot  [e, tgt] = g_e * 1[tgt_e]
                    aet = spool.tile([EPB, 128], bf16, tag="aet",
                                     name=f"aet{b}_{k}", bufs=3)
                    nc.vector.tensor_scalar_mul(
                        aet[:],
                        tgto_sb[:, (b * 2 + k) * 128:(b * 2 + k + 1) * 128],
                        gcol[:])
                    pre[k] = (pg, aet)
                pg2 = spool.tile([128, NPRED], bf16, tag="pg2",
                                 name=f"pg2_{b}", bufs=3)
                nc.vector.tensor_scalar_mul(pg2[:], p1hs_sb[:], g2_tiles[b][:])
                pre["pg2"] = pg2
                build_pre[b] = pre

            def build_b(b):
                """Stage B: dedup-compressed scatter matrices A_k and G^T.
                Consumes stage-A DVE outputs from the previous packet."""
                pre = build_pre.pop(b)
                pgt_t = psmall.tile([NPRED, 128], f32, tag="ps", name=f"pgt{b}")
                for k in range(2):
                    pg, aet = pre[k]
                    nc.tensor.matmul(
                        pgt_t[:], pg[:],
                        tgto_sb[:, (b * 2 + k) * 128:(b * 2 + k + 1) * 128],
                        start=(k == 0), stop=False)
                    # dedup-compress: A[us, tgt] = sum_{e: src_e=us} g_e 1[..]
                    u, boff = st["ub"][k][b], st["boff"][k][b]
                    pa_t = psmall.tile([u, 128], f32, tag="ps",
                                       name=f"pa{b}_{k}")
                    nc.tensor.matmul(pa_t[:],
                                     sdd_sb[k][:, boff:boff + u],
                                     aet[:], start=True, stop=True)
                    a_sb = apool.tile([u, 128], bf16, tag="a",
                                      name=f"a{b}_{k}")
                    nc.vector.tensor_copy(out=a_sb[:], in_=pa_t[:])
                    a_tiles[(b, k)] = a_sb
                # self-loop: G row 0 += g2
                nc.tensor.matmul(pgt_t[:], pre["pg2"][:], ident_sb[:],
                                 start=False, stop=True)
                gt_sb = gtpool.tile([NPRED, 128], bf16, tag="gt", name=f"gt{b}")
                nc.vector.tensor_copy(out=gt_sb[:], in_=pgt_t[:])
                gt_tiles[b] = gt_sb

            build_pre = {}

            # ---------------- phase 1: H2 + gf + gates ----------------
            for b in range(NBLK):
                if b + 2 < NBLK:
                    _load_rft(b + 2)
                if b == 1:
                    for m in range(4):
                        _load_w01(0, m)
                if b == 3:
                    for kc in range(8):
                        _load_xst(0, 0, kc)
                h2_pass(b)
                if b >= 2:
                    build_b(b - 2)
                if b >= 1:
                    build_a(b - 1)
                del rft_tiles[b]
            build_a(NBLK - 1)
            build_b(NBLK - 2)
            build_b(NBLK - 1)
            # w2 slots are dead now; w01 k=1 reuses them (WAR on last H2)
            del w2_ch
            for m in range(4):
                _load_w01(1, m)

            # ------------- phase 2: gathered k=0/1 streams -------------
            def stream_group(gi):
                k, g = allgroups[gi]
                blocks, goff, ug = st["groups"][k][g]
                pg_m = [pbig.tile([128, ug], f32, tag="pb",
                                  name=f"pgath{k}_{g}_{m}",
                                  padded_shape=[128, 512]) for m in range(4)]
                for kc in range(32):
                    # just-in-time prefetch, 8 tiles ahead (ring bufs=12)
                    pf = kc + 8
                    if pf < 32:
                        if (k, g, pf) not in xst_sb:
                            _load_xst(k, g, pf)
                    elif gi + 1 < len(allgroups):
                        nk, ng = allgroups[gi + 1]
                        if (nk, ng, pf - 32) not in xst_sb:
                            _load_xst(nk, ng, pf - 32)
                    xt = xst_sb[(k, g, kc)]
                    for m in range(4):
                        nc.tensor.matmul(
                            pg_m[m][:],
                            w01_sb[(k, m)][:, kc * 128:(kc + 1) * 128],
                            xt[:], start=(kc == 0), stop=(kc == 31))
                    del xst_sb[(k, g, kc)]
                hst_m = []
                for m in range(4):
                    hst = hstpool.tile([128, ug], bf16, tag="hst",
                                       name=f"hst{k}_{g}_{m}",
                                       padded_shape=[128, 512], bufs=10)
                    nc.vector.tensor_copy(out=hst[:], in_=pg_m[m][:])
                    hst_m.append(hst)
                return hst_m

            def transpose_group(gi, hst_m):
                k, g = allgroups[gi]
                blocks, goff, ug = st["groups"][k][g]
                for b in blocks:
                    u = st["ub"][k][b]
                    off = st["boff"][k][b] - goff
                    hs = hspool.tile([u, CW], bf16, tag="hs",
                                     name=f"hs{k}_{b}")
                    for m in range(4):
                        pt_t = psmall.tile([u, 128], bf16, tag="ps",
                                           name=f"pt{k}_{b}_{m}")
                        nc.tensor.transpose(
                            pt_t[:], hst_m[m][:, off:off + u], ident_sb[:])
                        nc.vector.tensor_copy(
                            out=hs[:, m * 128:(m + 1) * 128], in_=pt_t[:])
                    hs_tiles[(b, k)] = hs

            prev = None
            for gi in range(len(allgroups)):
                hst_m = stream_group(gi)
                if prev is not None:
                    transpose_group(*prev)
                prev = (gi, hst_m)
            transpose_group(*prev)

            # ------------- phase 3: scatter + bias + relu -------------
            for b in range(NBLK):
                pout_t = pbig.tile([128, CW], f32, tag="pb", name=f"po{b}")
                for k in range(2):
                    nc.tensor.matmul(pout_t[:], a_tiles[(b, k)][:],
                                     hs_tiles[(b, k)][:],
                                     start=(k == 0), stop=False)
                nc.tensor.matmul(pout_t[:], gt_tiles[b][:], blab_sb[:],
                                 start=False, stop=True)
                nc.vector.tensor_add(pout_t[:], pout_t[:], h2s_tiles[b][:])
                out_sb = opool.tile([128, CW], bf16, tag="out", name=f"ob{b}")
                nc.scalar.activation(out_sb[:], pout_t[:], AF.Relu)
                nc.sync.dma_start(out=out[b], in_=out_sb[:])
                del hs_tiles[(b, 0)], hs_tiles[(b, 1)]
                del a_tiles[(b, 0)], a_tiles[(b, 1)]
                del gt_tiles[b], h2s_tiles[b]
                del gf_tiles[b], g2_tiles[b]

    nc.compile()
    return nc


def _host_prep(inputs, st):
    rf = np.asarray(inputs["region_feats"], dtype=np.float32)
    W = np.asarray(inputs["W_conv"], dtype=np.float32)
    Wg = np.asarray(inputs["W_g"], dtype=np.float32)
    blab = np.asarray(inputs["b_lab"], dtype=np.float32)
    bglab = np.asarray(inputs["b_glab"], dtype=np.float32)

    # RF^T tiles: rft_h[b, p, d*128+j] = RF[b*128+j, d*128+p]
    rft_h = np.ascontiguousarray(
        rf.T.reshape(32, 128, NBLK, 128).transpose(2, 1, 0, 3), dtype=BF
    ).reshape(NBLK, 128, 32 * 128)

    # W2 per core: [p, kc*512+j] = W[kc*128+p, 2*D + c*512 + j]
    Wr = W.reshape(32, 128, 3, NCORES, CW)
    w2_cores = [
        np.ascontiguousarray(Wr[:, :, 2, c, :].transpose(1, 0, 2),
                             dtype=BF).reshape(128, 32 * CW)
        for c in range(NCORES)
    ]
    # W01 chunks: [p, ((k*4+m)*32+kc)*128+ch] = W[kc*128+p, k*D+c*512+m*128+ch]
    Wr2 = W.reshape(32, 128, 3, NCORES, 4, 128)
    w01_cores = [
        np.ascontiguousarray(Wr2[:, :, :2, c].transpose(1, 2, 3, 0, 4),
                             dtype=BF).reshape(128, 2 * 4 * 32 * 128)
        for c in range(NCORES)
    ]
    wg_h = np.ascontiguousarray(
        Wg.reshape(32, 128, 3).transpose(1, 0, 2), dtype=BF
    ).reshape(128, 32 * 3)
    blab_cores = [
        np.ascontiguousarray(blab[:, c * CW:(c + 1) * CW], dtype=BF)
        for c in range(NCORES)
    ]
    bgb_h = np.ascontiguousarray(
        np.repeat(bglab.reshape(1, NPRED), 128, axis=0), dtype=BF)

    srct_h = np.zeros((128, NBLK * 2 * EPB), np.float32)
    tgto_h = np.zeros((EPB, NBLK * 2 * 128), np.float32)
    p1h_h = np.zeros((EPB, NBLK * NPRED), np.float32)
    e = np.arange(EPB)
    xst_h, sdd_h = [], []
    for k in range(2):
        cols = []
        sdd = np.zeros((EPB, st["U"][k]), np.float32)
        for b in range(NBLK):
            src = st["src"][k][b]
            tgt = st["tgt"][k][b]
            srct_h[src, (b * 2 + k) * EPB + e] = 1.0
            tgto_h[e, (b * 2 + k) * 128 + tgt] = 1.0
            if k == 0:
                p1h_h[e, b * NPRED + st["pred"][b]] = 1.0
            sdd[e, st["boff"][k][b] + st["inv"][k][b]] = 1.0
            cols.append(rf[b * 128 + st["us"][k][b]])
        X = np.concatenate(cols, axis=0)            # [U, 4096]
        xst_h.append(np.ascontiguousarray(
            X.T.reshape(32, 128, st["U"][k]), dtype=BF))
        sdd_h.append(sdd.astype(BF))
    p1hs_h = np.zeros((128, NPRED), np.float32)
    p1hs_h[:, 0] = 1.0

    shared = {
        "rft": rft_h,
        "wg": wg_h,
        "bgb": bgb_h,
        "srct": srct_h.astype(BF),
        "tgto": tgto_h.astype(BF),
        "p1h": p1h_h.astype(BF),
        "p1hs": p1hs_h.astype(BF),
        "ident": np.eye(128, dtype=np.float32).astype(BF),
        "xst0": xst_h[0],
        "xst1": xst_h[1],
        "sdd0": sdd_h[0],
        "sdd1": sdd_h[1],
    }
    in_maps = []
    for c in range(NCORES):
        m = dict(shared)
        m["w2"] = w2_cores[c]
        m["w01"] = w01_cores[c]
        m["blab"] = blab_cores[c]
        in_maps.append(m)
    return in_maps


def _rels_are_blocked(rels):
    """Check each image's relations reference only that image's regions."""
    rels = np.asarray(rels)
    if rels.shape != (N_IMG * RPI, 3):
        return False
    rels_r = rels.reshape(N_IMG, RPI, 3)[:, :NUM_REL]
    img = np.arange(N_IMG)[:, None]
    lo, hi = img * REG, (img + 1) * REG
    so = rels_r[:, :, 1:3]
    return bool(np.all((so >= lo[:, :, None]) & (so < hi[:, :, None])))


def _numpy_fallback(inputs):
    """Reference-equivalent host computation (only used if the per-image
    relation structure assumption is violated)."""
    rf = np.asarray(inputs["region_feats"], dtype=np.float32)
    W = np.asarray(inputs["W_conv"], dtype=np.float32)
    Wg = np.asarray(inputs["W_g"], dtype=np.float32)
    blab = np.asarray(inputs["b_lab"], dtype=np.float32)
    bglab = np.asarray(inputs["b_glab"], dtype=np.float32)
    rels = np.asarray(inputs["rels"])
    preds = np.asarray(inputs["pred_classes"])
    rels_r = rels.reshape(N_IMG, RPI, 3)[:, :NUM_REL].reshape(-1, 3)
    preds_r = preds.reshape(N_IMG, RPI)[:, :NUM_REL].reshape(-1)
    nf = (rf @ W).reshape(-1, D)
    gfe = (rf @ Wg).reshape(-1)
    s, o = rels_r[:, 1], rels_r[:, 2]
    self_ids = np.arange(N)
    idx = np.concatenate([o * 3 + 0, s * 3 + 1, self_ids * 3 + 2])
    pr = np.concatenate([preds_r, preds_r, np.zeros(N, preds_r.dtype)])
    tgt = np.concatenate([s, o, self_ids])
    gate = 1.0 / (1.0 + np.exp(-(gfe[idx] + bglab[pr, 0])))
    msg = gate[:, None] * (nf[idx] + blab[pr])
    upd = np.zeros((N, D), np.float32)
    np.add.at(upd, tgt, msg)
    return np.maximum(upd, 0.0)


def _run(inputs, trace=False):
    from concourse.bass_utils import run_bass_kernel_spmd

    rels = np.asarray(inputs["rels"])
    preds = np.asarray(inputs["pred_classes"])
    key = (rels.tobytes(), preds.tobytes())
    if _prog_cache.get("key") != key:
        st = _structure(rels, preds)
        _prog_cache["nc"] = _build_program(st)
        _prog_cache["st"] = st
        _prog_cache["key"] = key
    nc = _prog_cache["nc"]
    in_maps = _host_prep(inputs, _prog_cache["st"])
    try:
        res = run_bass_kernel_spmd(nc, in_maps, core_ids=list(range(NCORES)),
                                   trace=trace)
    except Exception:
        # transient device errors (e.g. NRT_EXEC_UNIT_UNRECOVERABLE) have
        # been observed to clear on retry
        import time
        time.sleep(5)
        res = run_bass_kernel_spmd(nc, in_maps, core_ids=list(range(NCORES)),
                                   trace=trace)
    out = np.empty((N, D), np.float32)
    for c in range(NCORES):
        out[:, c * CW:(c + 1) * CW] = (
            np.asarray(res.results[c]["out"]).astype(np.float32).reshape(N, CW))
    return out, res


def kernel(**inputs):
    if not _rels_are_blocked(inputs["rels"]):
        return _numpy_fallback(inputs)
    out, _ = _run(inputs, trace=False)
    return out
